# revision 8
# baseline (speedup 1.0000x reference)
"""Trainium2 Bass kernel for the ConditionalMixturePrior GNN (8 NeuronCores).

Sharding: nodes contiguous (12500/core); edges by receiver-owner, ordered by
(source-chunk, receiver-window) so h[s] gathers use int16 dma_gather against
four static 25088-row slices of the replicated node table, h[r] gathers use
the core-local table, and segment-sum uses statically-scheduled one-hot
matmuls (deterministic). Node states replicated via AllGather each layer.
Compute bf16 with fp32 PSUM; LayerNorm via mean-projection matmul +
ACT Square / Abs_reciprocal_sqrt + K=1 broadcast matmul.
"""
import math
import sys

sys.path.insert(0, "/opt/trn_rl_repo")
import importlib.util as _ilu

import numpy as np

try:  # NTFF profile hook shim (harmless if unavailable)
    import antenv as _antenv
    if "antenv.axon_hooks" not in sys.modules:
        _sp = _ilu.spec_from_file_location(
            "antenv.axon_hooks", "/opt/trn_rl_repo/antenv/axon_hooks.py")
        if _sp is not None:
            _m = _ilu.module_from_spec(_sp)
            _sp.loader.exec_module(_m)
            sys.modules["antenv.axon_hooks"] = _m
            _antenv.axon_hooks = _m
except Exception:
    pass

import ml_dtypes
import concourse.bass as bass
import concourse.bass_isa as bass_isa
import concourse.bacc as bacc
import concourse.mybir as mybir
import concourse.tile as tile
from concourse.masks import make_identity
from concourse.bass_utils import run_bass_kernel_spmd

bf16 = mybir.dt.bfloat16
f32 = mybir.dt.float32
i16 = mybir.dt.int16
AF = mybir.ActivationFunctionType
ALU = mybir.AluOpType

N, E, B, H = 100000, 600000, 16, 128
IN_VAR, EDGE_VAR, ZD, KC, NMP = 12, 4, 32, 10, 3
MINLS, MAXLS, EPS = math.log(0.05), 5.0, 1e-5
NCORE = 8
NPC = N // NCORE
NODE_PAD = 12672           # 99*128, multiple of 512? no: 12672 = 24.75*512
NWIN = NODE_PAD // 128     # 99
CHUNK = 25088
HF_PAD = 100864            # 788*128 = 197*512
GPC = B // NCORE
bfnp = ml_dtypes.bfloat16


def _wrap16(idx_flat):
    n = idx_flat.shape[0]
    a = idx_flat.reshape(n // 16, 16).T
    return np.tile(a, (8, 1)).astype(np.int16)


def _prep(x, edge_attr, edge_index, batch):
    s_all = edge_index[0].astype(np.int64)
    r_all = edge_index[1].astype(np.int64)
    owner = r_all // NPC

    region = {}
    rcnt = np.zeros((NCORE, 4, NWIN), np.int64)
    for c in range(NCORE):
        eids = np.where(owner == c)[0]
        sc = (s_all[eids] // CHUNK).astype(np.int64)
        wl = ((r_all[eids] - c * NPC) // 128).astype(np.int64)
        for k in range(4):
            mk = sc == k
            selk, wk = eids[mk], wl[mk]
            o = np.argsort(wk, kind="stable")
            selk, wk = selk[o], wk[o]
            cuts = np.searchsorted(wk, np.arange(NWIN + 1))
            for w in range(NWIN):
                region[(c, k, w)] = selk[cuts[w]:cuts[w + 1]]
                rcnt[c, k, w] = cuts[w + 1] - cuts[w]
    rpad = ((rcnt.max(axis=0) + 127) // 128) * 128        # [4, NWIN]
    bsz = rpad.sum(axis=1)
    BPAD = int(((bsz.max() + 511) // 512) * 512)
    EPAD = 4 * BPAD
    TB = BPAD // 512
    TE = 4 * TB

    # static agg schedule: per bucket, chunk -> window (-1 = no-op)
    agg_sched = []
    for k in range(4):
        sk = []
        for w in range(NWIN):
            sk += [w] * int(rpad[k, w] // 128)
        sk += [-1] * (BPAD // 128 - len(sk))
        agg_sched.append(sk)

    bounds = np.searchsorted(batch, np.arange(B + 1))
    TP = HF_PAD // 512

    in_maps = []
    for c in range(NCORE):
        sidx = np.zeros((128, TE * 32), np.int16)
        ridx = np.zeros((128, TE * 32), np.int16)
        ea_pad = np.zeros((EPAD, EDGE_VAR), np.float32)
        rrel = np.full((128, TE * 4), -1e9, np.float32)
        for k in range(4):
            sl = np.zeros(BPAD, np.int16)
            rl = np.zeros(BPAD, np.int16)
            pos = 0
            for w in range(NWIN):
                sel = region[(c, k, w)]
                n = len(sel)
                sl[pos:pos + n] = (s_all[sel] - k * CHUNK).astype(np.int16)
                rl[pos:pos + n] = (r_all[sel] - c * NPC).astype(np.int16)
                ea_pad[k * BPAD + pos:k * BPAD + pos + n] = edge_attr[sel]
                npad = int(rpad[k, w])
                base_chunk = (k * BPAD + pos) // 128
                for j in range(npad // 128):
                    rr = rl[pos + j * 128:pos + (j + 1) * 128].astype(np.float32) - w * 128
                    lane = np.arange(j * 128, (j + 1) * 128)
                    rr[lane >= n] = -1e9
                    rrel[:, base_chunk + j] = rr
                pos += npad
            for t in range(TB):
                tid = k * TB + t
                sidx[:, tid * 32:(tid + 1) * 32] = _wrap16(sl[t * 512:(t + 1) * 512])
                ridx[:, tid * 32:(tid + 1) * 32] = _wrap16(rl[t * 512:(t + 1) * 512])

        xT = np.zeros((IN_VAR, NODE_PAD), bfnp)
        xT[:, :NPC] = x[c * NPC:(c + 1) * NPC].T.astype(bfnp)

        masks = np.zeros((128, GPC * TP * 4), np.float32)
        node_ids = np.arange(HF_PAD).reshape(TP, 4, 128)
        for q in range(GPC):
            g = GPC * c + q
            lo, hi = int(bounds[g]), int(bounds[g + 1])
            m = ((node_ids >= lo) & (node_ids < hi)).astype(np.float32)
            masks[:, q * TP * 4:(q + 1) * TP * 4] = m.transpose(2, 0, 1).reshape(128, TP * 4)

        in_maps.append({
            "xT": xT, "eaT": np.ascontiguousarray(ea_pad.T.astype(bfnp)),
            "sidx": sidx, "ridx": ridx, "rrel": rrel, "masks": masks,
        })

    plan = {"BPAD": BPAD, "EPAD": EPAD, "TB": TB, "TE": TE,
            "agg_sched": agg_sched, "TP": TP}
    return in_maps, plan


def _wprep(params):
    def mlpw(p, parts):
        w1 = np.asarray(p["w1"], np.float32)
        if parts > 1:
            w1p = [np.ascontiguousarray(w1[i * H:(i + 1) * H]).astype(bfnp)
                   for i in range(parts)]
        else:
            w1p = [np.ascontiguousarray(w1).astype(bfnp)]
        out = {"w1": w1p,
               "w2": np.ascontiguousarray(np.asarray(p["w2"], np.float32).astype(bfnp)),
               "b1": np.asarray(p["b1"], np.float32).reshape(H, 1),
               "b2": np.asarray(p["b2"], np.float32).reshape(H, 1)}
        if "g" in p:
            out["g"] = np.asarray(p["g"], np.float32).reshape(H, 1)
            out["beta"] = np.asarray(p["beta"], np.float32).reshape(H, 1)
        return out

    return {
        "ne": mlpw(params["node_enc"], 1),
        "ee": mlpw(params["edge_enc"], 1),
        "mp": [{"e": mlpw(l["edge"], 3), "n": mlpw(l["node"], 2)}
               for l in params["mp"]],
        "P": (np.eye(H) - 1.0 / H).astype(bfnp),
        "iota": np.tile(np.arange(128, dtype=np.float32), (128, 1)),
        "gw": np.ascontiguousarray(np.asarray(params["gate_w"], np.float32).astype(bfnp)),
        "hw1": np.ascontiguousarray(np.asarray(params["head"]["w1"], np.float32).astype(bfnp)),
        "hb1": np.asarray(params["head"]["b1"], np.float32).reshape(H, 1),
        "hw2": np.ascontiguousarray(np.asarray(params["head"]["w2"], np.float32).astype(bfnp)),
        "hb2": np.tile(np.asarray(params["head"]["b2"], np.float32)[None, :], (GPC, 1)),
    }


def build_kernel(plan, w, nmp=NMP):
    TE, TB, EPAD, TP = plan["TE"], plan["TB"], plan["EPAD"], plan["TP"]
    agg_sched = plan["agg_sched"]

    nc = bacc.Bacc("TRN2", target_bir_lowering=False, debug=False,
                   num_devices=NCORE)

    xT_d = nc.dram_tensor("xT", [IN_VAR, NODE_PAD], bf16, kind="ExternalInput")
    eaT_d = nc.dram_tensor("eaT", [EDGE_VAR, EPAD], bf16, kind="ExternalInput")
    sidx_d = nc.dram_tensor("sidx", [128, TE * 32], i16, kind="ExternalInput")
    ridx_d = nc.dram_tensor("ridx", [128, TE * 32], i16, kind="ExternalInput")
    rrel_d = nc.dram_tensor("rrel", [128, TE * 4], f32, kind="ExternalInput")
    masks_d = nc.dram_tensor("masks", [128, GPC * TP * 4], f32, kind="ExternalInput")
    out_d = nc.dram_tensor("out", [GPC, KC * 65], f32, kind="ExternalOutput")

    def cd(name, arr):
        return nc.inline_tensor(np.ascontiguousarray(arr), name=name)

    wt = {}
    mlps = [("ne", w["ne"]), ("ee", w["ee"])]
    for i in range(nmp):
        mlps += [(f"mp{i}e", w["mp"][i]["e"]), (f"mp{i}n", w["mp"][i]["n"])]
    for mk, mv in mlps:
        wt[mk] = {"w1": [cd(f"{mk}_w1_{j}", p) for j, p in enumerate(mv["w1"])],
                  "w2": cd(f"{mk}_w2", mv["w2"]),
                  "b1": cd(f"{mk}_b1", mv["b1"]),
                  "b2": cd(f"{mk}_b2", mv["b2"])}
        if "g" in mv:
            wt[mk]["g"] = cd(f"{mk}_g", mv["g"])
            wt[mk]["beta"] = cd(f"{mk}_beta", mv["beta"])
    P_d = cd("Pmat", w["P"])
    iota_d = cd("iota", w["iota"])
    gw_d = cd("gw", w["gw"])
    hw1_d = cd("hw1", w["hw1"])
    hb1_d = cd("hb1", w["hb1"])
    hw2_d = cd("hw2", w["hw2"])
    hb2_d = cd("hb2", w["hb2"])

    h_own = [nc.dram_tensor(f"h_own{i}", [NODE_PAD, H], bf16) for i in range(2)]
    h_full = [nc.dram_tensor(f"h_full{i}", [HF_PAD, H], bf16, addr_space="Shared")
              for i in range(2)]
    eT_dram = [nc.dram_tensor(f"eTd{i}", [H, EPAD], bf16) for i in range(2)]
    rg = [list(range(NCORE))]

    import contextlib
    with tile.TileContext(nc) as tc, contextlib.ExitStack() as ctx:
        pool = ctx.enter_context(tc.tile_pool(name="work", bufs=3))
        cpool = ctx.enter_context(tc.tile_pool(name="const", bufs=1))
        rows_pool = ctx.enter_context(tc.tile_pool(name="rows", bufs=2))
        ps_mlp = ctx.enter_context(tc.tile_pool(name="psmlp", bufs=2, space="PSUM"))
        ps_c = ctx.enter_context(tc.tile_pool(name="psc", bufs=2, space="PSUM"))
        ps_small = ctx.enter_context(tc.tile_pool(name="pssm", bufs=1, space="PSUM"))
        ps_agg = ctx.enter_context(tc.tile_pool(name="psagg", bufs=1, space="PSUM"))
        ps_t = ctx.enter_context(tc.tile_pool(name="pst", bufs=1, space="PSUM"))

        _cn = [0]
        def load_const(d, shape, dtype):
            _cn[0] += 1
            t = cpool.tile(shape, dtype, tag=f"c{_cn[0]}", name=f"c{_cn[0]}")
            nc.sync.dma_start(t[:], d[:])
            return t

        ws = {}
        for mk in wt:
            m = wt[mk]
            ws[mk] = {"w1": [load_const(t, list(t.shape), bf16) for t in m["w1"]],
                      "w2": load_const(m["w2"], [H, H], bf16),
                      "b1": load_const(m["b1"], [H, 1], f32),
                      "b2": load_const(m["b2"], [H, 1], f32)}
            if "g" in m:
                ws[mk]["g"] = load_const(m["g"], [H, 1], f32)
                ws[mk]["beta"] = load_const(m["beta"], [H, 1], f32)
        Pt = load_const(P_d, [H, H], bf16)
        iota_t = load_const(iota_d, [128, 128], f32)
        gw_t = load_const(gw_d, [H, 1], bf16)
        hw1_t = load_const(hw1_d, [H, H], bf16)
        hb1_t = load_const(hb1_d, [H, 1], f32)
        hw2_t = load_const(hw2_d, [H, KC * 65], bf16)
        hb2_t = load_const(hb2_d, [GPC, KC * 65], f32)
        sidx_t = load_const(sidx_d, [128, TE * 32], i16)
        ridx_t = load_const(ridx_d, [128, TE * 32], i16)
        rrel_t = load_const(rrel_d, [128, TE * 4], f32)
        masks_t = load_const(masks_d, [128, GPC * TP * 4], f32)

        ones_t = cpool.tile([H, 1], bf16)
        nc.vector.memset(ones_t[:], 1.0)
        one1_t = cpool.tile([1, H], bf16)
        nc.vector.memset(one1_t[:], 1.0)
        one11_t = cpool.tile([1, 1], bf16)
        nc.vector.memset(one11_t[:], 1.0)
        eps_t = cpool.tile([1, 1], f32)
        nc.vector.memset(eps_t[:], EPS)
        zero128 = cpool.tile([128, H], bf16)
        nc.vector.memset(zero128[:], 0.0)
        ident = cpool.tile([128, 128], f32)
        make_identity(nc, ident[:])
        ident16 = cpool.tile([128, 128], bf16)
        nc.vector.tensor_copy(ident16[:], ident[:])
        agg_sbuf = cpool.tile([H, NODE_PAD], f32)

        # zero h_full tail rows (pool pass reads them)
        ZLO = 99968  # 781*128
        for hf in h_full:
            nc.sync.dma_start(
                hf[ZLO:HF_PAD, :].rearrange("(o p) d -> p o d", p=128),
                zero128[:, None, :].to_broadcast([128, (HF_PAD - ZLO) // 128, H]))

        def mlp_tail(mk, y1_ps, M, resid, out_sb):
            m = ws[mk]
            h1 = pool.tile([H, 512], bf16, tag="h1")
            nc.scalar.activation(h1[:, :M], y1_ps, AF.Relu, bias=m["b1"][:])
            y2 = ps_mlp.tile([H, 512], f32, space="PSUM", tag="mlp")
            nc.tensor.matmul(y2[:, :M], lhsT=m["w2"][:], rhs=h1[:, :M],
                             start=True, stop=True)
            y2b = pool.tile([H, 512], bf16, tag="y2b")
            nc.scalar.activation(y2b[:, :M], y2[:, :M], AF.Identity, bias=m["b2"][:])
            c_ps = ps_c.tile([H, 512], f32, space="PSUM", tag="cps")
            nc.tensor.matmul(c_ps[:, :M], lhsT=Pt[:], rhs=y2b[:, :M],
                             start=True, stop=True)
            sq = pool.tile([H, 512], bf16, tag="sq")
            nc.scalar.activation(sq[:, :M], c_ps[:, :M], AF.Square)
            c_sb = pool.tile([H, 512], bf16, tag="csb")
            nc.scalar.activation(c_sb[:, :M], c_ps[:, :M], AF.Copy)
            v_ps = ps_small.tile([1, 512], f32, space="PSUM", tag="sm")
            nc.tensor.matmul(v_ps[:, :M], lhsT=ones_t[:], rhs=sq[:, :M],
                             start=True, stop=True)
            rstd = pool.tile([1, 512], bf16, tag="rstd")
            nc.scalar.activation(rstd[:, :M], v_ps[:, :M], AF.Abs_reciprocal_sqrt,
                                 bias=eps_t[:], scale=1.0 / H)
            R_ps = ps_c.tile([H, 512], f32, space="PSUM", tag="cps")
            nc.tensor.matmul(R_ps[:, :M], lhsT=one1_t[:], rhs=rstd[:, :M],
                             start=True, stop=True)
            norm = pool.tile([H, 512], bf16, tag="norm")
            nc.vector.tensor_tensor(out=norm[:, :M], in0=c_sb[:, :M],
                                    in1=R_ps[:, :M], op=ALU.mult)
            if resid is None:
                nc.scalar.activation(out_sb, norm[:, :M], AF.Identity,
                                     bias=m["beta"][:], scale=m["g"][:])
            else:
                gb = pool.tile([H, 512], bf16, tag="gbt")
                nc.scalar.activation(gb[:, :M], norm[:, :M], AF.Identity,
                                     bias=m["beta"][:], scale=m["g"][:])
                nc.vector.tensor_tensor(out=out_sb, in0=gb[:, :M], in1=resid,
                                        op=ALU.add)

        NTILES = (NODE_PAD + 511) // 512

        def node_pass(layer, cur):
            mk = "ne" if layer < 0 else f"mp{layer}n"
            nxt = 0 if layer < 0 else (cur + 1) % 2
            for t in range(NTILES):
                lo = t * 512
                M = min(512, NODE_PAD - lo)
                nchunk = M // 128
                if layer < 0:
                    xt = pool.tile([IN_VAR, 512], bf16, tag="xt")
                    nc.sync.dma_start(xt[:, :M], xT_d[:, lo:lo + M])
                    y1 = ps_mlp.tile([H, 512], f32, space="PSUM", tag="mlp")
                    nc.tensor.matmul(y1[:, :M], lhsT=ws[mk]["w1"][0][:],
                                     rhs=xt[:, :M], start=True, stop=True)
                    resid = None
                else:
                    hT = pool.tile([H, 512], bf16, tag="hT")
                    nc.sync.dma_start_transpose(hT[:, :M], h_own[cur][lo:lo + M, :])
                    agg16 = pool.tile([H, 512], bf16, tag="agg16")
                    nc.vector.tensor_copy(agg16[:, :M], agg_sbuf[:, lo:lo + M])
                    y1 = ps_mlp.tile([H, 512], f32, space="PSUM", tag="mlp")
                    nc.tensor.matmul(y1[:, :M], lhsT=ws[mk]["w1"][0][:],
                                     rhs=hT[:, :M], start=True, stop=False)
                    nc.tensor.matmul(y1[:, :M], lhsT=ws[mk]["w1"][1][:],
                                     rhs=agg16[:, :M], start=False, stop=True)
                    resid = hT[:, :M]
                hnew = pool.tile([H, 512], bf16, tag="hnew")
                mlp_tail(mk, y1[:, :M], M, resid, hnew[:, :M])
                rp = ps_t.tile([128, 4, 128], bf16, space="PSUM", tag="trp")
                for c4 in range(nchunk):
                    nc.tensor.transpose(rp[:, c4, :],
                                        hnew[:, c4 * 128:(c4 + 1) * 128], ident16[:])
                rows = rows_pool.tile([128, 4, 128], bf16, tag="rows")
                nc.vector.tensor_copy(rows[:, :nchunk, :], rp[:, :nchunk, :])
                nc.sync.dma_start(
                    h_own[nxt][lo:lo + M, :].rearrange("(i p) d -> p i d", p=128),
                    rows[:, :nchunk, :])

        def allgather(nxt):
            nc.gpsimd.collective_compute(
                "AllGather", ALU.bypass, replica_groups=rg,
                ins=[h_own[nxt][0:NPC, :]], outs=[h_full[nxt][0:N, :]])

        def edge_pass(layer, cur, ecur):
            mk = "ee" if layer < 0 else f"mp{layer}e"
            if layer >= 0:
                nc.vector.memset(agg_sbuf[:], 0.0)
            win_ps = {}
            for t in range(TE):
                lo = t * 512
                if layer < 0:
                    eat = pool.tile([EDGE_VAR, 512], bf16, tag="eat")
                    nc.sync.dma_start(eat[:], eaT_d[:, lo:lo + 512])
                    y1 = ps_mlp.tile([H, 512], f32, space="PSUM", tag="mlp")
                    nc.tensor.matmul(y1[:], lhsT=ws[mk]["w1"][0][:], rhs=eat[:],
                                     start=True, stop=True)
                    resid = None
                else:
                    k = t // TB
                    hsT = pool.tile([H, 1, 512], bf16, tag="hsT")
                    nc.gpsimd.dma_gather(
                        out_ap=hsT[:], in_ap=h_full[cur][k * CHUNK:(k + 1) * CHUNK, :],
                        idxs_ap=sidx_t[:, t * 32:(t + 1) * 32], num_idxs=512,
                        num_idxs_reg=512, elem_size=H, transpose=True)
                    hrT = pool.tile([H, 1, 512], bf16, tag="hrT")
                    nc.gpsimd.dma_gather(
                        out_ap=hrT[:], in_ap=h_own[cur][:, :],
                        idxs_ap=ridx_t[:, t * 32:(t + 1) * 32], num_idxs=512,
                        num_idxs_reg=512, elem_size=H, transpose=True)
                    eT_t = pool.tile([H, 512], bf16, tag="eTt")
                    nc.sync.dma_start(eT_t[:], eT_dram[ecur][:, lo:lo + 512])
                    y1 = ps_mlp.tile([H, 512], f32, space="PSUM", tag="mlp")
                    nc.tensor.matmul(y1[:], lhsT=ws[mk]["w1"][0][:], rhs=hsT[:, 0, :],
                                     start=True, stop=False)
                    nc.tensor.matmul(y1[:], lhsT=ws[mk]["w1"][1][:], rhs=hrT[:, 0, :],
                                     start=False, stop=False)
                    nc.tensor.matmul(y1[:], lhsT=ws[mk]["w1"][2][:], rhs=eT_t[:],
                                     start=False, stop=True)
                    resid = eT_t[:]
                enew = pool.tile([H, 512], bf16, tag="enew")
                mlp_tail(mk, y1[:], 512, resid, enew[:])
                nc.sync.dma_start(eT_dram[1 - ecur][:, lo:lo + 512], enew[:])
                if layer < 0:
                    continue
                rp = ps_t.tile([128, 4, 128], bf16, space="PSUM", tag="trp")
                for c4 in range(4):
                    nc.tensor.transpose(rp[:, c4, :],
                                        enew[:, c4 * 128:(c4 + 1) * 128], ident16[:])
                rows = rows_pool.tile([128, 4, 128], bf16, tag="rows")
                nc.vector.tensor_copy(rows[:], rp[:])
                kb = t // TB
                tb = t % TB
                for c4 in range(4):
                    ci = tb * 4 + c4
                    wv = agg_sched[kb][ci]
                    if wv < 0:
                        continue
                    gchunk = t * 4 + c4
                    oh = pool.tile([128, 128], bf16, tag="oh")
                    nc.vector.tensor_scalar(
                        out=oh[:], in0=iota_t[:],
                        scalar1=rrel_t[:, gchunk:gchunk + 1], scalar2=None,
                        op0=ALU.is_equal)
                    start = (ci == 0) or (agg_sched[kb][ci - 1] != wv)
                    fin = (ci == TB * 4 - 1) or (agg_sched[kb][ci + 1] != wv)
                    if start:
                        wnew = ps_agg.tile([128, 128], f32, space="PSUM",
                                           tag="aggw", name=f"aggw_{t}_{c4}")
                        win_ps[wv] = wnew
                    wtile = win_ps[wv]
                    nc.tensor.matmul(wtile[:], lhsT=rows[:, c4, :], rhs=oh[:],
                                     start=start, stop=fin)
                    if fin:
                        nc.vector.tensor_tensor(
                            out=agg_sbuf[:, wv * 128:(wv + 1) * 128],
                            in0=agg_sbuf[:, wv * 128:(wv + 1) * 128],
                            in1=wtile[:], op=ALU.add)

        def pool_head(cur):
            hf = h_full[cur]
            gate_sb = cpool.tile([128, TP * 4], f32)
            for t in range(TP):
                lo = t * 512
                hT = pool.tile([H, 512], bf16, tag="phT")
                nc.sync.dma_start_transpose(hT[:], hf[lo:lo + 512, :])
                g_ps = ps_small.tile([1, 512], f32, space="PSUM", tag="sm")
                nc.tensor.matmul(g_ps[:], lhsT=gw_t[:], rhs=hT[:], start=True, stop=True)
                g16 = pool.tile([1, 512], bf16, tag="g16")
                nc.scalar.activation(g16[:], g_ps[:], AF.Copy)
                gt_ps = ps_t.tile([128, 4], f32, space="PSUM", tag="gtp")
                for c4 in range(4):
                    nc.tensor.matmul(gt_ps[:, c4:c4 + 1],
                                     lhsT=g16[:, c4 * 128:(c4 + 1) * 128],
                                     rhs=one11_t[:], start=True, stop=True)
                nc.vector.tensor_copy(gate_sb[:, t * 4:(t + 1) * 4], gt_ps[:])
            pooledT = cpool.tile([H, GPC], bf16)
            for q in range(GPC):
                mq = masks_t[:, q * TP * 4:(q + 1) * TP * 4]
                mneg = pool.tile([128, TP * 4], f32, tag="mneg")
                nc.vector.tensor_scalar(out=mneg[:], in0=mq, scalar1=1.0,
                                        scalar2=1e30, op0=ALU.subtract,
                                        op1=ALU.mult)
                gq = pool.tile([128, TP * 4], f32, tag="gq")
                nc.vector.tensor_tensor(out=gq[:], in0=gate_sb[:], in1=mneg[:],
                                        op=ALU.add)
                gmax = pool.tile([128, 1], f32, tag="gmax")
                nc.vector.tensor_reduce(out=gmax[:], in_=gq[:],
                                        axis=mybir.AxisListType.X, op=ALU.max)
                gmax_a = pool.tile([128, 1], f32, tag="gmaxa")
                nc.gpsimd.partition_all_reduce(gmax_a[:], gmax[:], channels=128,
                                               reduce_op=bass_isa.ReduceOp.max)
                ngmax = pool.tile([128, 1], f32, tag="ngmax")
                nc.vector.tensor_scalar_mul(ngmax[:], gmax_a[:], -1.0)
                wq = pool.tile([128, TP * 4], f32, tag="wq")
                nc.scalar.activation(wq[:], gq[:], AF.Exp, bias=ngmax[:])
                den = pool.tile([128, 1], f32, tag="den")
                nc.vector.tensor_reduce(out=den[:], in_=wq[:],
                                        axis=mybir.AxisListType.X, op=ALU.add)
                den_a = pool.tile([128, 1], f32, tag="dena")
                nc.gpsimd.partition_all_reduce(den_a[:], den[:], channels=128,
                                               reduce_op=bass_isa.ReduceOp.add)
                rden = pool.tile([128, 1], f32, tag="rden")
                nc.vector.reciprocal(rden[:], den_a[:])
                attn = pool.tile([128, TP * 4], bf16, tag="attn")
                nc.vector.tensor_scalar_mul(attn[:], wq[:], rden[:])
                p_ps = ps_small.tile([1, 512], f32, space="PSUM", tag="sm")
                for t in range(TP):
                    hr = rows_pool.tile([128, 4, 128], bf16, tag="prows")
                    nc.sync.dma_start(
                        hr[:], hf[t * 512:(t + 1) * 512, :]
                        .rearrange("(i p) d -> p i d", p=128))
                    for c4 in range(4):
                        nc.tensor.matmul(
                            p_ps[:, :H], lhsT=attn[:, t * 4 + c4:t * 4 + c4 + 1],
                            rhs=hr[:, c4, :],
                            start=(t == 0 and c4 == 0),
                            stop=(t == TP - 1 and c4 == 3))
                p16 = pool.tile([1, H], bf16, tag="p16")
                nc.scalar.activation(p16[:], p_ps[:, :H], AF.Copy)
                pT_ps = ps_t.tile([128, 4], f32, space="PSUM", tag="gtp")
                nc.tensor.matmul(pT_ps[:, 0:1], lhsT=p16[:], rhs=one11_t[:],
                                 start=True, stop=True)
                nc.vector.tensor_copy(pooledT[:, q:q + 1], pT_ps[:, 0:1])
            hy1 = ps_t.tile([128, 4], f32, space="PSUM", tag="gtp")
            nc.tensor.matmul(hy1[:, :GPC], lhsT=hw1_t[:], rhs=pooledT[:],
                             start=True, stop=True)
            hh = pool.tile([H, GPC], bf16, tag="hh")
            nc.scalar.activation(hh[:], hy1[:, :GPC], AF.Relu, bias=hb1_t[:])
            raw = pool.tile([GPC, KC * 65], f32, tag="raw")
            for j in range(2):
                lo = j * 325
                r_ps = ps_t.tile([2, 325], f32, space="PSUM", tag="gtp")
                nc.tensor.matmul(r_ps[:], lhsT=hh[:], rhs=hw2_t[:, lo:lo + 325],
                                 start=True, stop=True)
                nc.vector.tensor_tensor(out=raw[:, lo:lo + 325], in0=r_ps[:],
                                        in1=hb2_t[:, lo:lo + 325], op=ALU.add)
            rawv = raw[:].rearrange("g (k d) -> g k d", k=KC)
            nc.vector.tensor_scalar(out=rawv[:, :, 33:65], in0=rawv[:, :, 33:65],
                                    scalar1=MAXLS, scalar2=MINLS,
                                    op0=ALU.min, op1=ALU.max)
            nc.sync.dma_start(out_d[:], raw[:])

        edge_pass(-1, 0, 1)      # encoder writes eT_dram[0]
        node_pass(-1, 0)
        allgather(0)
        cur = 0
        ecur = 0
        for l in range(nmp):
            edge_pass(l, cur, ecur)  # reads eT[ecur], writes eT[1-ecur]
            ecur = 1 - ecur
            node_pass(l, cur)
            nxt = (cur + 1) % 2
            allgather(nxt)
            cur = nxt
        pool_head(cur)

    nc.finalize()
    return nc


_CACHE = {}


def kernel(x, edge_attr, edge_index, batch, params):
    x = np.asarray(x, np.float32)
    edge_attr = np.asarray(edge_attr, np.float32)
    edge_index = np.asarray(edge_index, np.int32)
    batch = np.asarray(batch, np.int32)

    in_maps, plan = _prep(x, edge_attr, edge_index, batch)
    w = _wprep(params)

    key = (plan["EPAD"], plan["TP"])
    if key not in _CACHE:
        _CACHE[key] = build_kernel(plan, w)
    nc = _CACHE[key]

    res = run_bass_kernel_spmd(nc, in_maps, core_ids=list(range(NCORE)))
    outs = [np.asarray(res.results[c]["out"]) for c in range(NCORE)]
    raw = np.concatenate(outs, axis=0).reshape(B, KC, 65)
    logits = raw[:, :, 0].astype(np.float32)
    mu = raw[:, :, 1:1 + ZD].astype(np.float32)
    log_std = raw[:, :, 1 + ZD:].astype(np.float32)
    return logits, mu, log_std


# revision 14
# speedup vs baseline: 1.0119x; 1.0119x over previous
"""Trainium2 Bass kernel for the ConditionalMixturePrior GNN (8 NeuronCores).

Sharding: nodes contiguous (12500/core); edges by receiver-owner, ordered by
(source-chunk, receiver-window) so h[s] gathers use int16 dma_gather against
four static 25088-row slices of the replicated node table, h[r] gathers use
the core-local table, and segment-sum uses statically-scheduled one-hot
matmuls (deterministic). Node states replicated via AllGather each layer.
Compute bf16 with fp32 PSUM; LayerNorm via mean-projection matmul +
ACT Square / Abs_reciprocal_sqrt + K=1 broadcast matmul.
"""
import math
import sys

sys.path.insert(0, "/opt/trn_rl_repo")
import importlib.util as _ilu

import numpy as np

try:  # NTFF profile hook shim (harmless if unavailable)
    import antenv as _antenv
    if "antenv.axon_hooks" not in sys.modules:
        _sp = _ilu.spec_from_file_location(
            "antenv.axon_hooks", "/opt/trn_rl_repo/antenv/axon_hooks.py")
        if _sp is not None:
            _m = _ilu.module_from_spec(_sp)
            _sp.loader.exec_module(_m)
            sys.modules["antenv.axon_hooks"] = _m
            _antenv.axon_hooks = _m
except Exception:
    pass

import ml_dtypes
import concourse.bass as bass
import concourse.bass_isa as bass_isa
import concourse.bacc as bacc
import concourse.mybir as mybir
import concourse.tile as tile
from concourse.masks import make_identity
from concourse.bass_utils import run_bass_kernel_spmd

bf16 = mybir.dt.bfloat16
f32 = mybir.dt.float32
i16 = mybir.dt.int16
AF = mybir.ActivationFunctionType
ALU = mybir.AluOpType

N, E, B, H = 100000, 600000, 16, 128
IN_VAR, EDGE_VAR, ZD, KC, NMP = 12, 4, 32, 10, 3
MINLS, MAXLS, EPS = math.log(0.05), 5.0, 1e-5
NCORE = 8
NPC = N // NCORE
NODE_PAD = 12672           # 99*128, multiple of 512? no: 12672 = 24.75*512
NWIN = NODE_PAD // 128     # 99
CHUNK = 25088
HF_PAD = 100864            # 788*128 = 197*512
GPC = B // NCORE
bfnp = ml_dtypes.bfloat16


def _wrap16(idx_flat):
    n = idx_flat.shape[0]
    a = idx_flat.reshape(n // 16, 16).T
    return np.tile(a, (8, 1)).astype(np.int16)


def _prep(x, edge_attr, edge_index, batch):
    s_all = edge_index[0].astype(np.int64)
    r_all = edge_index[1].astype(np.int64)
    owner = r_all // NPC

    region = {}
    rcnt = np.zeros((NCORE, 4, NWIN), np.int64)
    for c in range(NCORE):
        eids = np.where(owner == c)[0]
        sc = (s_all[eids] // CHUNK).astype(np.int64)
        wl = ((r_all[eids] - c * NPC) // 128).astype(np.int64)
        for k in range(4):
            mk = sc == k
            selk, wk = eids[mk], wl[mk]
            o = np.argsort(wk, kind="stable")
            selk, wk = selk[o], wk[o]
            cuts = np.searchsorted(wk, np.arange(NWIN + 1))
            for w in range(NWIN):
                region[(c, k, w)] = selk[cuts[w]:cuts[w + 1]]
                rcnt[c, k, w] = cuts[w + 1] - cuts[w]
    rpad = ((rcnt.max(axis=0) + 127) // 128) * 128        # [4, NWIN]
    bsz = rpad.sum(axis=1)
    BPAD = int(((bsz.max() + 511) // 512) * 512)
    EPAD = 4 * BPAD
    TB = BPAD // 512
    TE = 4 * TB

    # static agg schedule: per bucket, chunk -> window (-1 = no-op)
    agg_sched = []
    for k in range(4):
        sk = []
        for w in range(NWIN):
            sk += [w] * int(rpad[k, w] // 128)
        sk += [-1] * (BPAD // 128 - len(sk))
        agg_sched.append(sk)

    bounds = np.searchsorted(batch, np.arange(B + 1))
    TP = HF_PAD // 512

    in_maps = []
    for c in range(NCORE):
        sidx = np.zeros((128, TE * 32), np.int16)
        ridx = np.zeros((128, TE * 32), np.int16)
        ea_pad = np.zeros((EPAD, EDGE_VAR), np.float32)
        rrel = np.full((128, TE * 4), -1e9, np.float32)
        for k in range(4):
            sl = np.zeros(BPAD, np.int16)
            rl = np.zeros(BPAD, np.int16)
            pos = 0
            for w in range(NWIN):
                sel = region[(c, k, w)]
                n = len(sel)
                sl[pos:pos + n] = (s_all[sel] - k * CHUNK).astype(np.int16)
                rl[pos:pos + n] = (r_all[sel] - c * NPC).astype(np.int16)
                ea_pad[k * BPAD + pos:k * BPAD + pos + n] = edge_attr[sel]
                npad = int(rpad[k, w])
                base_chunk = (k * BPAD + pos) // 128
                for j in range(npad // 128):
                    rr = rl[pos + j * 128:pos + (j + 1) * 128].astype(np.float32) - w * 128
                    lane = np.arange(j * 128, (j + 1) * 128)
                    rr[lane >= n] = -1e9
                    rrel[:, base_chunk + j] = rr
                pos += npad
            for t in range(TB):
                tid = k * TB + t
                sidx[:, tid * 32:(tid + 1) * 32] = _wrap16(sl[t * 512:(t + 1) * 512])
                ridx[:, tid * 32:(tid + 1) * 32] = _wrap16(rl[t * 512:(t + 1) * 512])

        xT = np.zeros((IN_VAR, NODE_PAD), bfnp)
        xT[:, :NPC] = x[c * NPC:(c + 1) * NPC].T.astype(bfnp)

        masks = np.zeros((128, GPC * TP * 4), np.float32)
        node_ids = np.arange(HF_PAD).reshape(TP, 4, 128)
        for q in range(GPC):
            g = GPC * c + q
            lo, hi = int(bounds[g]), int(bounds[g + 1])
            m = ((node_ids >= lo) & (node_ids < hi)).astype(np.float32)
            masks[:, q * TP * 4:(q + 1) * TP * 4] = m.transpose(2, 0, 1).reshape(128, TP * 4)

        in_maps.append({
            "xT": xT, "eaT": np.ascontiguousarray(ea_pad.T.astype(bfnp)),
            "sidx": sidx, "ridx": ridx, "rrel": rrel, "masks": masks,
        })

    plan = {"BPAD": BPAD, "EPAD": EPAD, "TB": TB, "TE": TE,
            "agg_sched": agg_sched, "TP": TP}
    return in_maps, plan


def _wprep(params):
    def mlpw(p, parts):
        w1 = np.asarray(p["w1"], np.float32)
        if parts > 1:
            w1p = [np.ascontiguousarray(w1[i * H:(i + 1) * H]).astype(bfnp)
                   for i in range(parts)]
        else:
            w1p = [np.ascontiguousarray(w1).astype(bfnp)]
        out = {"w1": w1p,
               "w2": np.ascontiguousarray(np.asarray(p["w2"], np.float32).astype(bfnp)),
               "b1": np.asarray(p["b1"], np.float32).reshape(H, 1),
               "b2": np.asarray(p["b2"], np.float32).reshape(H, 1)}
        if "g" in p:
            out["g"] = np.asarray(p["g"], np.float32).reshape(H, 1)
            out["beta"] = np.asarray(p["beta"], np.float32).reshape(H, 1)
        return out

    return {
        "ne": mlpw(params["node_enc"], 1),
        "ee": mlpw(params["edge_enc"], 1),
        "mp": [{"e": mlpw(l["edge"], 3), "n": mlpw(l["node"], 2)}
               for l in params["mp"]],
        "P": (np.eye(H) - 1.0 / H).astype(bfnp),
        "iota": np.tile(np.arange(128, dtype=np.float32), (128, 1)),
        "gw": np.ascontiguousarray(np.asarray(params["gate_w"], np.float32).astype(bfnp)),
        "hw1": np.ascontiguousarray(np.asarray(params["head"]["w1"], np.float32).astype(bfnp)),
        "hb1": np.asarray(params["head"]["b1"], np.float32).reshape(H, 1),
        "hw2": np.ascontiguousarray(np.asarray(params["head"]["w2"], np.float32).astype(bfnp)),
        "hb2": np.tile(np.asarray(params["head"]["b2"], np.float32)[None, :], (GPC, 1)),
    }


def build_kernel(plan, w, nmp=NMP):
    TE, TB, EPAD, TP = plan["TE"], plan["TB"], plan["EPAD"], plan["TP"]
    agg_sched = plan["agg_sched"]

    nc = bacc.Bacc("TRN2", target_bir_lowering=False, debug=False,
                   num_devices=NCORE)

    xT_d = nc.dram_tensor("xT", [IN_VAR, NODE_PAD], bf16, kind="ExternalInput")
    eaT_d = nc.dram_tensor("eaT", [EDGE_VAR, EPAD], bf16, kind="ExternalInput")
    sidx_d = nc.dram_tensor("sidx", [128, TE * 32], i16, kind="ExternalInput")
    ridx_d = nc.dram_tensor("ridx", [128, TE * 32], i16, kind="ExternalInput")
    rrel_d = nc.dram_tensor("rrel", [128, TE * 4], f32, kind="ExternalInput")
    masks_d = nc.dram_tensor("masks", [128, GPC * TP * 4], f32, kind="ExternalInput")
    out_d = nc.dram_tensor("out", [GPC, KC * 65], f32, kind="ExternalOutput")

    def cd(name, arr):
        return nc.inline_tensor(np.ascontiguousarray(arr), name=name)

    wt = {}
    mlps = [("ne", w["ne"]), ("ee", w["ee"])]
    for i in range(nmp):
        mlps += [(f"mp{i}e", w["mp"][i]["e"]), (f"mp{i}n", w["mp"][i]["n"])]
    for mk, mv in mlps:
        wt[mk] = {"w1": [cd(f"{mk}_w1_{j}", p) for j, p in enumerate(mv["w1"])],
                  "w2": cd(f"{mk}_w2", mv["w2"]),
                  "b1": cd(f"{mk}_b1", mv["b1"]),
                  "b2": cd(f"{mk}_b2", mv["b2"])}
        if "g" in mv:
            wt[mk]["g"] = cd(f"{mk}_g", mv["g"])
            wt[mk]["beta"] = cd(f"{mk}_beta", mv["beta"])
    P_d = cd("Pmat", w["P"])
    iota_d = cd("iota", w["iota"])
    gw_d = cd("gw", w["gw"])
    hw1_d = cd("hw1", w["hw1"])
    hb1_d = cd("hb1", w["hb1"])
    hw2_d = cd("hw2", w["hw2"])
    hb2_d = cd("hb2", w["hb2"])

    h_own = [nc.dram_tensor(f"h_own{i}", [NODE_PAD, H], bf16) for i in range(2)]
    h_full = [nc.dram_tensor(f"h_full{i}", [HF_PAD, H], bf16, addr_space="Shared")
              for i in range(2)]
    eT_dram = [nc.dram_tensor(f"eTd{i}", [H, EPAD], bf16) for i in range(2)]
    rg = [list(range(NCORE))]

    import contextlib
    with tile.TileContext(nc) as tc, contextlib.ExitStack() as ctx:
        pool = ctx.enter_context(tc.tile_pool(name="work", bufs=4))
        cpool = ctx.enter_context(tc.tile_pool(name="const", bufs=1))
        rows_pool = ctx.enter_context(tc.tile_pool(name="rows", bufs=2))
        ppool = ctx.enter_context(tc.tile_pool(name="poolp", bufs=1))
        ps_mlp = ctx.enter_context(tc.tile_pool(name="psmlp", bufs=2, space="PSUM"))
        ps_c = ctx.enter_context(tc.tile_pool(name="psc", bufs=2, space="PSUM"))
        ps_small = ctx.enter_context(tc.tile_pool(name="pssm", bufs=1, space="PSUM"))
        ps_agg = ctx.enter_context(tc.tile_pool(name="psagg", bufs=1, space="PSUM"))
        ps_t = ctx.enter_context(tc.tile_pool(name="pst", bufs=1, space="PSUM"))

        _cn = [0]
        def load_const(d, shape, dtype):
            _cn[0] += 1
            t = cpool.tile(shape, dtype, tag=f"c{_cn[0]}", name=f"c{_cn[0]}")
            nc.sync.dma_start(t[:], d[:])
            return t

        ws = {}
        for mk in wt:
            m = wt[mk]
            ws[mk] = {"w1": [load_const(t, list(t.shape), bf16) for t in m["w1"]],
                      "w2": load_const(m["w2"], [H, H], bf16),
                      "b1": load_const(m["b1"], [H, 1], f32),
                      "b2": load_const(m["b2"], [H, 1], f32)}
            if "g" in m:
                ws[mk]["g"] = load_const(m["g"], [H, 1], f32)
                ws[mk]["beta"] = load_const(m["beta"], [H, 1], f32)
        Pt = load_const(P_d, [H, H], bf16)
        iota_t = load_const(iota_d, [128, 128], f32)
        gw_t = load_const(gw_d, [H, 1], bf16)
        hw1_t = load_const(hw1_d, [H, H], bf16)
        hb1_t = load_const(hb1_d, [H, 1], f32)
        hw2_t = load_const(hw2_d, [H, KC * 65], bf16)
        hb2_t = load_const(hb2_d, [GPC, KC * 65], f32)
        sidx_t = load_const(sidx_d, [128, TE * 32], i16)
        ridx_t = load_const(ridx_d, [128, TE * 32], i16)
        rrel_t = load_const(rrel_d, [128, TE * 4], f32)
        masks_t = load_const(masks_d, [128, GPC * TP * 4], f32)

        ones_t = cpool.tile([H, 1], bf16)
        nc.vector.memset(ones_t[:], 1.0)
        one1_t = cpool.tile([1, H], bf16)
        nc.vector.memset(one1_t[:], 1.0)
        one11_t = cpool.tile([1, 1], bf16)
        nc.vector.memset(one11_t[:], 1.0)
        eps_t = cpool.tile([1, 1], f32)
        nc.vector.memset(eps_t[:], EPS)
        zero128 = cpool.tile([128, H], bf16)
        nc.vector.memset(zero128[:], 0.0)
        ident = cpool.tile([128, 128], f32)
        make_identity(nc, ident[:])
        ident16 = cpool.tile([128, 128], bf16)
        nc.vector.tensor_copy(ident16[:], ident[:])
        agg_sbuf = cpool.tile([H, NODE_PAD], f32)

        # zero h_full tail rows (pool pass reads them)
        ZLO = 99968  # 781*128
        for hf in h_full:
            nc.sync.dma_start(
                hf[ZLO:HF_PAD, :].rearrange("(o p) d -> p o d", p=128),
                zero128[:, None, :].to_broadcast([128, (HF_PAD - ZLO) // 128, H]))

        def mlp_tail(mk, y1_ps, M, resid, out_sb):
            m = ws[mk]
            h1 = pool.tile([H, 512], bf16, tag="h1")
            nc.scalar.activation(h1[:, :M], y1_ps, AF.Relu, bias=m["b1"][:])
            y2 = ps_mlp.tile([H, 512], f32, space="PSUM", tag="mlp")
            nc.tensor.matmul(y2[:, :M], lhsT=m["w2"][:], rhs=h1[:, :M],
                             start=True, stop=True)
            y2b = pool.tile([H, 512], bf16, tag="y2b")
            nc.scalar.activation(y2b[:, :M], y2[:, :M], AF.Identity, bias=m["b2"][:])
            c_ps = ps_c.tile([H, 512], f32, space="PSUM", tag="cps")
            nc.tensor.matmul(c_ps[:, :M], lhsT=Pt[:], rhs=y2b[:, :M],
                             start=True, stop=True)
            sq = pool.tile([H, 512], bf16, tag="sq")
            nc.scalar.activation(sq[:, :M], c_ps[:, :M], AF.Square)
            c_sb = pool.tile([H, 512], bf16, tag="csb")
            nc.scalar.activation(c_sb[:, :M], c_ps[:, :M], AF.Copy)
            v_ps = ps_small.tile([1, 512], f32, space="PSUM", tag="sm")
            nc.tensor.matmul(v_ps[:, :M], lhsT=ones_t[:], rhs=sq[:, :M],
                             start=True, stop=True)
            rstd = pool.tile([1, 512], bf16, tag="rstd")
            nc.scalar.activation(rstd[:, :M], v_ps[:, :M], AF.Abs_reciprocal_sqrt,
                                 bias=eps_t[:], scale=1.0 / H)
            R_ps = ps_c.tile([H, 512], f32, space="PSUM", tag="cps")
            nc.tensor.matmul(R_ps[:, :M], lhsT=one1_t[:], rhs=rstd[:, :M],
                             start=True, stop=True)
            norm = pool.tile([H, 512], bf16, tag="norm")
            nc.vector.tensor_tensor(out=norm[:, :M], in0=c_sb[:, :M],
                                    in1=R_ps[:, :M], op=ALU.mult)
            if resid is None:
                nc.scalar.activation(out_sb, norm[:, :M], AF.Identity,
                                     bias=m["beta"][:], scale=m["g"][:])
            else:
                gb = pool.tile([H, 512], bf16, tag="gbt")
                nc.scalar.activation(gb[:, :M], norm[:, :M], AF.Identity,
                                     bias=m["beta"][:], scale=m["g"][:])
                nc.vector.tensor_tensor(out=out_sb, in0=gb[:, :M], in1=resid,
                                        op=ALU.add)

        NTILES = (NODE_PAD + 511) // 512

        def node_pass(layer, cur):
            mk = "ne" if layer < 0 else f"mp{layer}n"
            nxt = 0 if layer < 0 else (cur + 1) % 2
            for t in range(NTILES):
                lo = t * 512
                M = min(512, NODE_PAD - lo)
                nchunk = M // 128
                if layer < 0:
                    xt = pool.tile([IN_VAR, 512], bf16, tag="xt")
                    nc.sync.dma_start(xt[:, :M], xT_d[:, lo:lo + M])
                    y1 = ps_mlp.tile([H, 512], f32, space="PSUM", tag="mlp")
                    nc.tensor.matmul(y1[:, :M], lhsT=ws[mk]["w1"][0][:],
                                     rhs=xt[:, :M], start=True, stop=True)
                    resid = None
                else:
                    hT = pool.tile([H, 512], bf16, tag="hT")
                    nc.sync.dma_start_transpose(hT[:, :M], h_own[cur][lo:lo + M, :])
                    agg16 = pool.tile([H, 512], bf16, tag="agg16")
                    nc.vector.tensor_copy(agg16[:, :M], agg_sbuf[:, lo:lo + M])
                    y1 = ps_mlp.tile([H, 512], f32, space="PSUM", tag="mlp")
                    nc.tensor.matmul(y1[:, :M], lhsT=ws[mk]["w1"][0][:],
                                     rhs=hT[:, :M], start=True, stop=False)
                    nc.tensor.matmul(y1[:, :M], lhsT=ws[mk]["w1"][1][:],
                                     rhs=agg16[:, :M], start=False, stop=True)
                    resid = hT[:, :M]
                hnew = pool.tile([H, 512], bf16, tag="hnew")
                mlp_tail(mk, y1[:, :M], M, resid, hnew[:, :M])
                rp = ps_t.tile([128, 4, 128], bf16, space="PSUM", tag="trp")
                for c4 in range(nchunk):
                    nc.tensor.transpose(rp[:, c4, :],
                                        hnew[:, c4 * 128:(c4 + 1) * 128], ident16[:])
                rows = rows_pool.tile([128, 4, 128], bf16, tag="rows")
                nc.scalar.activation(rows[:, :nchunk, :], rp[:, :nchunk, :], AF.Copy)
                nc.sync.dma_start(
                    h_own[nxt][lo:lo + M, :].rearrange("(i p) d -> p i d", p=128),
                    rows[:, :nchunk, :])

        def allgather(nxt):
            nc.gpsimd.collective_compute(
                "AllGather", ALU.bypass, replica_groups=rg,
                ins=[h_own[nxt][0:NPC, :]], outs=[h_full[nxt][0:N, :]])

        def edge_pass(layer, cur, ecur):
            mk = "ee" if layer < 0 else f"mp{layer}e"
            if layer >= 0:
                nc.vector.memset(agg_sbuf[:], 0.0)
            win_ps = {}
            for t in range(TE):
                lo = t * 512
                if layer < 0:
                    eat = pool.tile([EDGE_VAR, 512], bf16, tag="eat")
                    nc.sync.dma_start(eat[:], eaT_d[:, lo:lo + 512])
                    y1 = ps_mlp.tile([H, 512], f32, space="PSUM", tag="mlp")
                    nc.tensor.matmul(y1[:], lhsT=ws[mk]["w1"][0][:], rhs=eat[:],
                                     start=True, stop=True)
                    resid = None
                else:
                    k = t // TB
                    hsT = pool.tile([H, 1, 512], bf16, tag="hsT")
                    nc.gpsimd.dma_gather(
                        out_ap=hsT[:], in_ap=h_full[cur][k * CHUNK:(k + 1) * CHUNK, :],
                        idxs_ap=sidx_t[:, t * 32:(t + 1) * 32], num_idxs=512,
                        num_idxs_reg=512, elem_size=H, transpose=True)
                    hrT = pool.tile([H, 1, 512], bf16, tag="hrT")
                    nc.gpsimd.dma_gather(
                        out_ap=hrT[:], in_ap=h_own[cur][:, :],
                        idxs_ap=ridx_t[:, t * 32:(t + 1) * 32], num_idxs=512,
                        num_idxs_reg=512, elem_size=H, transpose=True)
                    eT_t = pool.tile([H, 512], bf16, tag="eTt")
                    nc.sync.dma_start(eT_t[:], eT_dram[ecur][:, lo:lo + 512])
                    y1 = ps_mlp.tile([H, 512], f32, space="PSUM", tag="mlp")
                    nc.tensor.matmul(y1[:], lhsT=ws[mk]["w1"][0][:], rhs=hsT[:, 0, :],
                                     start=True, stop=False)
                    nc.tensor.matmul(y1[:], lhsT=ws[mk]["w1"][1][:], rhs=hrT[:, 0, :],
                                     start=False, stop=False)
                    nc.tensor.matmul(y1[:], lhsT=ws[mk]["w1"][2][:], rhs=eT_t[:],
                                     start=False, stop=True)
                    resid = eT_t[:]
                enew = pool.tile([H, 512], bf16, tag="enew")
                mlp_tail(mk, y1[:], 512, resid, enew[:])
                nc.sync.dma_start(eT_dram[1 - ecur][:, lo:lo + 512], enew[:])
                if layer < 0:
                    continue
                rp = ps_t.tile([128, 4, 128], bf16, space="PSUM", tag="trp")
                for c4 in range(4):
                    nc.tensor.transpose(rp[:, c4, :],
                                        enew[:, c4 * 128:(c4 + 1) * 128], ident16[:])
                rows = rows_pool.tile([128, 4, 128], bf16, tag="rows")
                nc.scalar.activation(rows[:], rp[:], AF.Copy)
                kb = t // TB
                tb = t % TB
                for c4 in range(4):
                    ci = tb * 4 + c4
                    wv = agg_sched[kb][ci]
                    if wv < 0:
                        continue
                    gchunk = t * 4 + c4
                    oh = pool.tile([128, 128], bf16, tag="oh")
                    nc.vector.tensor_scalar(
                        out=oh[:], in0=iota_t[:],
                        scalar1=rrel_t[:, gchunk:gchunk + 1], scalar2=None,
                        op0=ALU.is_equal)
                    start = (ci == 0) or (agg_sched[kb][ci - 1] != wv)
                    fin = (ci == TB * 4 - 1) or (agg_sched[kb][ci + 1] != wv)
                    if start:
                        wnew = ps_agg.tile([128, 128], f32, space="PSUM",
                                           tag="aggw", name=f"aggw_{t}_{c4}")
                        win_ps[wv] = wnew
                    wtile = win_ps[wv]
                    nc.tensor.matmul(wtile[:], lhsT=rows[:, c4, :], rhs=oh[:],
                                     start=start, stop=fin)
                    if fin:
                        nc.vector.tensor_tensor(
                            out=agg_sbuf[:, wv * 128:(wv + 1) * 128],
                            in0=agg_sbuf[:, wv * 128:(wv + 1) * 128],
                            in1=wtile[:], op=ALU.add)

        def pool_head(cur):
            hf = h_full[cur]
            gate_sb = cpool.tile([128, TP * 4], f32)
            for t in range(TP):
                lo = t * 512
                hT = pool.tile([H, 512], bf16, tag="phT")
                nc.sync.dma_start_transpose(hT[:], hf[lo:lo + 512, :])
                g_ps = ps_small.tile([1, 512], f32, space="PSUM", tag="sm")
                nc.tensor.matmul(g_ps[:], lhsT=gw_t[:], rhs=hT[:], start=True, stop=True)
                g16 = pool.tile([1, 512], bf16, tag="g16")
                nc.scalar.activation(g16[:], g_ps[:], AF.Copy)
                gt_ps = ps_t.tile([128, 4], f32, space="PSUM", tag="gtp")
                for c4 in range(4):
                    nc.tensor.matmul(gt_ps[:, c4:c4 + 1],
                                     lhsT=g16[:, c4 * 128:(c4 + 1) * 128],
                                     rhs=one11_t[:], start=True, stop=True)
                nc.vector.tensor_copy(gate_sb[:, t * 4:(t + 1) * 4], gt_ps[:])
            pooledT = cpool.tile([H, GPC], bf16)
            for q in range(GPC):
                mq = masks_t[:, q * TP * 4:(q + 1) * TP * 4]
                mneg = ppool.tile([128, TP * 4], f32, tag="mneg")
                nc.vector.tensor_scalar(out=mneg[:], in0=mq, scalar1=1.0,
                                        scalar2=1e30, op0=ALU.subtract,
                                        op1=ALU.mult)
                gq = ppool.tile([128, TP * 4], f32, tag="gq")
                nc.vector.tensor_tensor(out=gq[:], in0=gate_sb[:], in1=mneg[:],
                                        op=ALU.add)
                gmax = pool.tile([128, 1], f32, tag="gmax")
                nc.vector.tensor_reduce(out=gmax[:], in_=gq[:],
                                        axis=mybir.AxisListType.X, op=ALU.max)
                gmax_a = pool.tile([128, 1], f32, tag="gmaxa")
                nc.gpsimd.partition_all_reduce(gmax_a[:], gmax[:], channels=128,
                                               reduce_op=bass_isa.ReduceOp.max)
                ngmax = pool.tile([128, 1], f32, tag="ngmax")
                nc.vector.tensor_scalar_mul(ngmax[:], gmax_a[:], -1.0)
                wq = ppool.tile([128, TP * 4], f32, tag="wq")
                nc.scalar.activation(wq[:], gq[:], AF.Exp, bias=ngmax[:])
                den = pool.tile([128, 1], f32, tag="den")
                nc.vector.tensor_reduce(out=den[:], in_=wq[:],
                                        axis=mybir.AxisListType.X, op=ALU.add)
                den_a = pool.tile([128, 1], f32, tag="dena")
                nc.gpsimd.partition_all_reduce(den_a[:], den[:], channels=128,
                                               reduce_op=bass_isa.ReduceOp.add)
                rden = pool.tile([128, 1], f32, tag="rden")
                nc.vector.reciprocal(rden[:], den_a[:])
                attn = ppool.tile([128, TP * 4], bf16, tag="attn")
                nc.vector.tensor_scalar_mul(attn[:], wq[:], rden[:])
                p_ps = ps_small.tile([1, 512], f32, space="PSUM", tag="sm")
                for t in range(TP):
                    hr = rows_pool.tile([128, 4, 128], bf16, tag="prows")
                    nc.sync.dma_start(
                        hr[:], hf[t * 512:(t + 1) * 512, :]
                        .rearrange("(i p) d -> p i d", p=128))
                    for c4 in range(4):
                        nc.tensor.matmul(
                            p_ps[:, :H], lhsT=attn[:, t * 4 + c4:t * 4 + c4 + 1],
                            rhs=hr[:, c4, :],
                            start=(t == 0 and c4 == 0),
                            stop=(t == TP - 1 and c4 == 3))
                p16 = pool.tile([1, H], bf16, tag="p16")
                nc.scalar.activation(p16[:], p_ps[:, :H], AF.Copy)
                pT_ps = ps_t.tile([128, 4], f32, space="PSUM", tag="gtp")
                nc.tensor.matmul(pT_ps[:, 0:1], lhsT=p16[:], rhs=one11_t[:],
                                 start=True, stop=True)
                nc.vector.tensor_copy(pooledT[:, q:q + 1], pT_ps[:, 0:1])
            hy1 = ps_t.tile([128, 4], f32, space="PSUM", tag="gtp")
            nc.tensor.matmul(hy1[:, :GPC], lhsT=hw1_t[:], rhs=pooledT[:],
                             start=True, stop=True)
            hh = pool.tile([H, GPC], bf16, tag="hh")
            nc.scalar.activation(hh[:], hy1[:, :GPC], AF.Relu, bias=hb1_t[:])
            raw = pool.tile([GPC, KC * 65], f32, tag="raw")
            for j in range(2):
                lo = j * 325
                r_ps = ps_t.tile([2, 325], f32, space="PSUM", tag="gtp")
                nc.tensor.matmul(r_ps[:], lhsT=hh[:], rhs=hw2_t[:, lo:lo + 325],
                                 start=True, stop=True)
                nc.vector.tensor_tensor(out=raw[:, lo:lo + 325], in0=r_ps[:],
                                        in1=hb2_t[:, lo:lo + 325], op=ALU.add)
            rawv = raw[:].rearrange("g (k d) -> g k d", k=KC)
            nc.vector.tensor_scalar(out=rawv[:, :, 33:65], in0=rawv[:, :, 33:65],
                                    scalar1=MAXLS, scalar2=MINLS,
                                    op0=ALU.min, op1=ALU.max)
            nc.sync.dma_start(out_d[:], raw[:])

        edge_pass(-1, 0, 1)      # encoder writes eT_dram[0]
        node_pass(-1, 0)
        allgather(0)
        cur = 0
        ecur = 0
        for l in range(nmp):
            edge_pass(l, cur, ecur)  # reads eT[ecur], writes eT[1-ecur]
            ecur = 1 - ecur
            node_pass(l, cur)
            nxt = (cur + 1) % 2
            allgather(nxt)
            cur = nxt
        pool_head(cur)

    nc.finalize()
    return nc


_CACHE = {}


def kernel(x, edge_attr, edge_index, batch, params):
    x = np.asarray(x, np.float32)
    edge_attr = np.asarray(edge_attr, np.float32)
    edge_index = np.asarray(edge_index, np.int32)
    batch = np.asarray(batch, np.int32)

    in_maps, plan = _prep(x, edge_attr, edge_index, batch)
    w = _wprep(params)

    key = (plan["EPAD"], plan["TP"])
    if key not in _CACHE:
        _CACHE[key] = build_kernel(plan, w)
    nc = _CACHE[key]

    res = run_bass_kernel_spmd(nc, in_maps, core_ids=list(range(NCORE)))
    outs = [np.asarray(res.results[c]["out"]) for c in range(NCORE)]
    raw = np.concatenate(outs, axis=0).reshape(B, KC, 65)
    logits = raw[:, :, 0].astype(np.float32)
    mu = raw[:, :, 1:1 + ZD].astype(np.float32)
    log_std = raw[:, :, 1 + ZD:].astype(np.float32)
    return logits, mu, log_std
